# revision 7
# baseline (speedup 1.0000x reference)
"""LoRA QKV projection kernel for Trainium2 (Bass/Tile), 8-core SPMD.

Problem: x [B=4, S=2048, D=4096] fp32; for each of q/k/v:
    out = x @ W.T + (x @ A.T) @ B.T      (W [H=4096, D], A [R=16, D], B [H, R])

Sharding: data-parallel over tokens. Each of the 8 cores owns 1024 of the
8192 tokens and computes all 3*4096 output columns for them; weights are
replicated.

Host-side prep folds the rank-16 LoRA update into the dense weight
(W' = W + B@A, exact in fp32 -- standard merged-LoRA deployment) and casts
x / W' to bf16, so the device program is a pure bf16 GEMM with fp32 PSUM
accumulation:  out[T,3H] = x[T,D] @ W'[D,3H].

Device loop (per core): x.T resident in SBUF ([128,T] bf16 per 128-row
d-block); W' streamed column-chunk by column-chunk (24 chunks of 512 cols,
each 32 d-tiles of [128,512] bf16) with ~2.5 chunks of DMA prefetch on the
SP queue while x/output DMAs ride the Activation queue. Each chunk runs
32x8 PE matmuls accumulating into all 8 PSUM banks; the DVE drains banks
to SBUF and outputs stream back to HBM. The PE never waits: weights are
prefetched, bank drains complete in the 7-matmul shadow after each bank's
last accumulation, keeping the tensor engine at its top p-state.

bf16 inputs with fp32 accumulation give ~1e-3 max-abs rel err (vs the 2e-2
gate): quantization is 2^-9 RMS per operand, and errors stay relative under
the K=4096 random-sign accumulation.
"""

import sys
import types

import numpy as np
import ml_dtypes

import concourse.bass as bass
import concourse.mybir as mybir
import concourse.tile as tile
from concourse import bacc, bass_utils


def _install_profiling_shim():
    """Make trace=True usable under axon on images whose ``antenv`` lacks
    ``axon_hooks``: inject the module and register the ctypes NTFF hook.
    Harmless no-op when the real module exists. Also keep profile artifacts
    local (no bucket upload is available here)."""
    try:
        if "antenv.axon_hooks" not in sys.modules:
            try:
                from antenv import axon_hooks  # noqa: F401
            except ImportError:
                mod = types.ModuleType("antenv.axon_hooks")
                mod._hook = None
                mod.set_axon_ntff_profile_hook = lambda h: setattr(
                    mod, "_hook", h)
                mod.get_axon_ntff_profile_hook = lambda: mod._hook
                sys.modules["antenv.axon_hooks"] = mod
                import antenv
                antenv.axon_hooks = mod
                try:
                    from trn_agent_boot.trn_boot import _ntff_profile_via_ctypes
                    hook = _ntff_profile_via_ctypes("/opt/axon/libaxon_pjrt.so")
                    if hook is not None:
                        mod.set_axon_ntff_profile_hook(hook)
                except Exception:
                    pass
        bass_utils.upload_artifacts = lambda tmpdir: "local://" + str(tmpdir)
    except Exception:
        pass


_install_profiling_shim()

F32 = mybir.dt.float32
BF16 = mybir.dt.bfloat16
NP_BF16 = ml_dtypes.bfloat16

N_CORES = 8
P = 128          # partition dim
NCH = 512        # matmul moving free dim / psum bank width (fp32)


def _build(D, T, H, n_cores=N_CORES):
    """Build the per-core Bass program.

    D: model dim (contraction), T: tokens per core, H: output columns per
    projection. All multiples of the tile sizes used below.
    """
    DT = D // P                # d-tiles (32)
    ST = T // P                # token tiles per chunk == psum banks (8)
    CH_PER_PROJ = H // NCH
    NCHUNK = 3 * CH_PER_PROJ   # h-chunks across q,k,v (24)

    assert ST <= 8, "token tiles must fit in the 8 psum banks"

    nc = bacc.Bacc("TRN2", target_bir_lowering=False, debug=False,
                   num_devices=n_cores)

    # Host-pretiled layouts (contiguous per DMA):
    #   xt [DT, 128, T]      bf16 : xt[dt, p, t] = x_core[t, dt*128+p]
    #   wt [NCHUNK, DT, 128, NCH] bf16 : wt[j, dt, p, n] = W'.T[dt*128+p, j*512+n]
    xt_d = nc.dram_tensor("xt", [DT, P, T], BF16, kind="ExternalInput")
    wt_d = nc.dram_tensor("wt", [NCHUNK, DT, P, NCH], BF16,
                          kind="ExternalInput")
    outs_d = [
        nc.dram_tensor(name, [T, H], F32, kind="ExternalOutput")
        for name in ("q", "k", "v")
    ]

    with tile.TileContext(nc) as tc:
        with (
            tc.tile_pool(name="xp", bufs=DT) as xp,
            tc.tile_pool(name="wp", bufs=80) as wp,
            tc.tile_pool(name="psum", bufs=8, space="PSUM") as psum,
            tc.tile_pool(name="outp", bufs=8) as outp,
        ):
            # ---- x load. The SP queue is reserved for the weight stream so
            # chunk 0's weights land immediately; x rides the other three
            # queues. The first tiles are split into halves across queues so
            # the PE's chunk-0 d-loop is never starved at startup. ----
            xqs = [nc.scalar, nc.gpsimd]
            xt = []
            for dt in range(DT):
                t = xp.tile([P, T], BF16, tag="x", name=f"x_{dt}")
                if dt < 4:
                    q = T // 4
                    for i in range(4):
                        xqs[(dt + i) % 2].dma_start(
                            t[:, i * q:(i + 1) * q],
                            xt_d[dt, :, i * q:(i + 1) * q])
                elif dt < 8:
                    h = T // 2
                    xqs[dt % 2].dma_start(t[:, :h], xt_d[dt, :, :h])
                    xqs[(dt + 1) % 2].dma_start(t[:, h:], xt_d[dt, :, h:])
                else:
                    xqs[dt % 2].dma_start(t[:], xt_d[dt])
                xt.append(t)

            # ---- main loop: stream W' chunks, accumulate in psum banks ----
            for j in range(NCHUNK):
                pj, hoff = j // CH_PER_PROJ, (j % CH_PER_PROJ) * NCH
                ps = [psum.tile([P, NCH], F32, tag="ps", name=f"ps_{j}_{s}")
                      for s in range(ST)]
                wt = []
                for dt in range(DT):
                    w = wp.tile([P, NCH], BF16, tag="w", name=f"w_{j}_{dt}")
                    nc.sync.dma_start(w[:], wt_d[j, dt])
                    wt.append(w)
                oqs = [nc.scalar, nc.gpsimd, nc.sync]
                if j < NCHUNK - 1:
                    for dt in range(DT):
                        for s in range(ST):
                            nc.tensor.matmul(
                                ps[s],
                                xt[dt][:, s * P:(s + 1) * P],
                                wt[dt][:],
                                start=(dt == 0),
                                stop=(dt == DT - 1),
                            )
                    for s in range(ST):
                        ot = outp.tile([P, NCH], F32, tag="o")
                        nc.vector.tensor_copy(ot[:], ps[s])
                        # rotate output queues so bursts of output traffic
                        # spread over the DMA dispatchers
                        oqs[s % 3].dma_start(
                            outs_d[pj][s * P:(s + 1) * P, hoff:hoff + NCH],
                            ot[:],
                        )
                else:
                    # Last chunk: s-outer so banks close ~6.8us apart and
                    # drains + output DMAs overlap the remaining matmuls
                    # instead of serializing after the final one.
                    for s in range(ST):
                        for dt in range(DT):
                            nc.tensor.matmul(
                                ps[s],
                                xt[dt][:, s * P:(s + 1) * P],
                                wt[dt][:],
                                start=(dt == 0),
                                stop=(dt == DT - 1),
                            )
                        ot = outp.tile([P, NCH], F32, tag="o")
                        nc.vector.tensor_copy(ot[:], ps[s])
                        h = NCH // 2
                        dst = outs_d[pj][s * P:(s + 1) * P, hoff:hoff + NCH]
                        oqs[s % 3].dma_start(dst[:, :h], ot[:, :h])
                        oqs[(s + 1) % 3].dma_start(dst[:, h:], ot[:, h:])

    nc.compile()
    return nc


_NC_CACHE = {}


def _get_nc(D, T, H):
    key = (D, T, H)
    if key not in _NC_CACHE:
        _NC_CACHE[key] = _build(D, T, H)
    return _NC_CACHE[key]


def _run(x, q_weight, k_weight, v_weight, q_A, q_B, k_A, k_B, v_A, v_B,
         trace=False):
    Bb, S, D = x.shape
    H = q_weight.shape[0]
    TOK = Bb * S
    T = TOK // N_CORES
    DT = D // P
    NCHUNK = 3 * (H // NCH)

    nc = _get_nc(D, T, H)

    # Fold LoRA into the dense weights (exact, fp32): W'.T = W.T + A.T @ B.T
    wT = np.empty((D, 3 * H), dtype=np.float32)
    for i, (w, a, b) in enumerate(
            ((q_weight, q_A, q_B), (k_weight, k_A, k_B), (v_weight, v_A, v_B))):
        w = np.asarray(w, dtype=np.float32)
        a = np.asarray(a, dtype=np.float32)
        b = np.asarray(b, dtype=np.float32)
        wT[:, i * H:(i + 1) * H] = w.T
        wT[:, i * H:(i + 1) * H] += a.T @ b.T
    # bf16, pretiled: [NCHUNK, DT, P, NCH]
    wt = np.ascontiguousarray(
        wT.astype(NP_BF16).reshape(DT, P, NCHUNK, NCH).transpose(2, 0, 1, 3))

    xbf = np.asarray(x, dtype=np.float32).reshape(TOK, D).astype(NP_BF16)
    in_maps = []
    for c in range(N_CORES):
        xc = np.ascontiguousarray(
            xbf[c * T:(c + 1) * T, :].T.reshape(DT, P, T))
        in_maps.append({"xt": xc, "wt": wt})

    res = bass_utils.run_bass_kernel_spmd(
        nc, in_maps, core_ids=list(range(N_CORES)), trace=trace)

    full = []
    for name in ("q", "k", "v"):
        full.append(
            np.concatenate([res.results[c][name] for c in range(N_CORES)],
                           axis=0).reshape(Bb, S, H))
    return tuple(full), res


def kernel(**inputs):
    out, _ = _run(**inputs)
    return out
